# revision 19
# baseline (speedup 1.0000x reference)
# Trainium2 Bass kernel for masked causal attention
#   B=2, H=16, S=2048, D=64, bool attn_mask [B, S, S] + causal, softmax, @V.
#
# Sharding: 8 cores x 4 heads (cores 0-3 -> batch 0, cores 4-7 -> batch 1).
# Each core computes its 4 heads fully on-device; the per-batch mask is
# resident in SBUF and shared by the core's 4 heads.
#
# Per (head, k-tile j of 128 keys):
#   S^T[k, q] = sum_d K[k,d] Q[q,d]     (PE: lhsT=K^T tile, rhs=Q^T, fp16)
#   p[k, q]   = exp(S^T/8) * mask^T     (ACT exp from PSUM -> fp16 SBUF; DVE mult)
#   outT[m,q] += sum_k vp[k,m] p[k,q]   (PE: lhsT=[ones | V] -> row 0 = denom)
# then outT[1:65]/denom via DVE reciprocal (straight from PSUM row 0) +
# DRAM-roundtrip DMA partition-broadcast + DVE mult, fp16 out DMA.
#
# Structure notes (performance):
# - causal trim everywhere: k-tile j only computes q >= 128j, and the mask
#   DMA only loads those columns (DMA saturation degrades the PE clock).
# - outp is split into two [65, 1024] halves (2 PSUM banks each, pool
#   bufs=2): the A half (cols < 1024) takes PV only from j <= 7, finishes
#   mid-head, and frees its banks early so the next head's PV never stalls
#   on a full-head drain.
# - PV work is queued and drained in dense bursts (the PE p-state decays
#   to 1.2 GHz unless it gets long gapless runs).
# - NOTE: ROW_TILE (paired row-group QK on disjoint PE row halves) hangs
#   the device (NRT_EXEC_UNIT_UNRECOVERABLE) despite passing CoreSim.

import numpy as np

B, H, S, D = 2, 16, 2048, 64
NCORES = 8
HPC = 4          # heads per core
P = 128
NKT = S // P     # 16 k-tiles
CHUNK = 1024     # S^T psum tile width (2 PSUM banks; bufs=2 double-buffers)
ST_BUFS = 2
HALF = 1024      # outp half width
VOFF = 64        # V block offset in vp: [ones | pad(63) | V] -> numerator
                 # starts at PSUM partition 64 (a 64-partition DVE access
                 # must begin at partition 0 or 64)
VPW = VOFF + D   # vp width (96)
PVA_LAG = 2      # prompt queue: PV-A trails QK by this many k-tiles
MASK_PACE = False
WARM_N = 16      # warm-up matmuls (clock ramp cover for the input DMA window)
PVB_HIGH = 8     # lazy queue: drain in bursts of PVB_BURST above this
PVB_BURST = 4
NORM_DEFER = 2   # norm mult flushes this many PV emissions after its chain

_cache = {}


def build_nc():
    import concourse.bacc as bacc
    import concourse.mybir as mybir
    import concourse.tile as tile
    from contextlib import ExitStack

    fp16 = mybir.dt.float16
    f32 = mybir.dt.float32
    Exp = mybir.ActivationFunctionType.Exp

    nc = bacc.Bacc("TRN2", target_bir_lowering=False, debug=False,
                   num_devices=NCORES)

    qt_d = nc.dram_tensor("qt", [HPC, 64, S], fp16, kind="ExternalInput")
    kt_d = nc.dram_tensor("kt", [HPC, 64, S], fp16, kind="ExternalInput")
    vp_d = nc.dram_tensor("vp", [HPC, P, NKT, D + 1], fp16, kind="ExternalInput")
    mk_d = nc.dram_tensor("maskt", [P, NKT, S], fp16, kind="ExternalInput")
    out_d = nc.dram_tensor("outt", [HPC, D, S], fp16, kind="ExternalOutput")

    with tile.TileContext(nc) as tc, ExitStack() as ctx:
        mask_pool = ctx.enter_context(tc.tile_pool(name="mask", bufs=1))
        qk_pool = ctx.enter_context(tc.tile_pool(name="qk", bufs=2))
        vp_pool = ctx.enter_context(tc.tile_pool(name="vpool", bufs=2))
        p_pool = ctx.enter_context(tc.tile_pool(name="p", bufs=max(12, PVB_HIGH + 4)))
        o_pool = ctx.enter_context(tc.tile_pool(name="osb", bufs=3))
        r_pool = ctx.enter_context(tc.tile_pool(name="recip", bufs=4))
        warm_pool = ctx.enter_context(tc.tile_pool(name="warm", bufs=1))
        rb_pool = ctx.enter_context(tc.tile_pool(name="rb", bufs=4))
        st_psum = ctx.enter_context(tc.tile_pool(name="st", bufs=ST_BUFS, space="PSUM"))
        o_psum = ctx.enter_context(tc.tile_pool(name="outp", bufs=2, space="PSUM"))
        dram_pool = ctx.enter_context(tc.tile_pool(name="dram", bufs=4, space="DRAM"))

        # PE warm-up: dense back-to-back matmuls on zeros right at kernel
        # start so the clock-gate opens to 2.4 GHz before the real QK stream.
        wsb = warm_pool.tile([P, 512], fp16, tag="warm")
        nc.vector.memset(wsb[:], 0.0)
        wps = st_psum.tile([P, CHUNK], f32, tag="st")
        for i in range(WARM_N):
            lo = 512 * (i % 2)
            nc.tensor.matmul(wps[:, lo:lo + 512], lhsT=wsb[:, 0:128],
                             rhs=wsb[:], start=True, stop=True)

        def load_head(h):
            qt = qk_pool.tile([64, S], fp16, tag="qt")
            nc.sync.dma_start(qt[:], qt_d[h])
            kt = qk_pool.tile([64, S], fp16, tag="kt")
            nc.sync.dma_start(kt[:], kt_d[h])
            vp = vp_pool.tile([P, NKT, VPW], fp16, tag="vp")
            # ones col -> 0, V -> 64:128; pad cols 1:64 stay garbage (they
            # only feed PSUM partitions 1:63, which are never read)
            nc.sync.dma_start(vp[:, :, 0:1], vp_d[h, :, :, 0:1])
            nc.sync.dma_start(vp[:, :, VOFF:VPW], vp_d[h, :, :, 1:D + 1])
            return qt, kt, vp

        # Head 0 inputs first (unblocks the first QK ~4us in), then the big
        # per-batch mask^T streams in behind it, causally trimmed.
        head_tiles = {0: load_head(0)}
        mask_sb = mask_pool.tile([P, NKT, S], fp16, tag="mask")

        def load_mask(g):
            nc.sync.dma_start(mask_sb[:, g:g + 1, g * P:],
                              mk_d[:, g:g + 1, g * P:])

        for g in range(8 if MASK_PACE else NKT):
            load_mask(g)

        pvA = []          # (h, outA, vp, j, pj) — prompt, j <= 7 only
        pvB = []          # (h, outB, vp, j, pj) — lazy burst reserve
        norm_pend = []    # (due_emit_count, h, outX, s0, rbc)
        n_emitted = [0]

        def flush_norms(force=False):
            while norm_pend and (force or norm_pend[0][0] <= n_emitted[0]):
                _, h, outX, s0, rbc = norm_pend.pop(0)
                lo = s0 % HALF
                osb = o_pool.tile([D, 512], fp16, tag="osb")
                nc.vector.tensor_mul(osb[:], outX[VOFF:VOFF + D, lo:lo + 512], rbc[:])
                nc.sync.dma_start(out_d[h, :, s0:s0 + 512], osb[:])

        def emit_norm(h, outX, s0):
            # denominator is row 0 of outX (ones-first vp), read from PSUM
            # directly (base partition 0, so reciprocal_approx_fast is safe).
            lo = s0 % HALF
            recip = r_pool.tile([1, 512], f32, tag="recip")
            nc.vector.reciprocal_approx_fast(out=recip[0:1, :],
                                             in_=outX[0:1, lo:lo + 512])
            # partition-broadcast via a DRAM round trip (2 small DMAs):
            # avoids the gpsimd library entirely (its TENSOR_LOAD serializes
            # ~2us of kernel startup) and keeps the Pool engine idle.
            rscr = dram_pool.tile([1, 512], f32, tag="rscr")
            nc.sync.dma_start(rscr[0:1, :], recip[0:1, :])
            rbc = rb_pool.tile([D, 512], f32, tag="rbc")
            nc.sync.dma_start(rbc[:], rscr[0:1, :].to_broadcast((D, 512)))
            norm_pend.append((n_emitted[0] + NORM_DEFER, h, outX, s0, rbc))

        def emit_pvA(h, outA, vp, j, pj):
            flush_norms()
            c = j * P
            for b in range(c // 512, 2):
                g0, g1 = max(c, 512 * b), 512 * (b + 1)
                nc.tensor.matmul(outA[:, g0:g1], lhsT=vp[:, j, :],
                                 rhs=pj[:, g0 - c:g1 - c],
                                 start=(j == 0),
                                 stop=(j == min(4 * b + 3, 7)))
            n_emitted[0] += 1
            if j == 3:
                emit_norm(h, outA, 0)
            elif j == 7:
                emit_norm(h, outA, 512)

        def emit_pvB(h, outB, vp, j, pj):
            if j == 0:
                # outB reuses the previous head's B banks; its norm mults
                # must be on the DVE queue before the PE waits on the slot.
                flush_norms(force=True)
            else:
                flush_norms()
            c = j * P
            for b in range(max(c - HALF, 0) // 512, 2):
                g0 = max(c, HALF + 512 * b)
                g1 = HALF + 512 * (b + 1)
                nc.tensor.matmul(outB[:, g0 - HALF:g1 - HALF],
                                 lhsT=vp[:, j, :], rhs=pj[:, g0 - c:g1 - c],
                                 start=(j == 0),
                                 stop=(j == min(4 * b + 11, NKT - 1)))
            n_emitted[0] += 1
            if j == 11:
                emit_norm(h, outB, HALF)
            elif j == 15:
                emit_norm(h, outB, HALF + 512)

        def chunks(j):
            out, c = [], j * P
            while c < S:
                e = min(S, (c // CHUNK + 1) * CHUNK)
                out.append((c, e))
                c = e
            return out

        for h in range(HPC):
            qt, kt, vp = head_tiles.pop(h, None) or load_head(h)
            outA = o_psum.tile([VPW, HALF], f32, tag="outp")
            outB = o_psum.tile([VPW, HALF], f32, tag="outp")

            for j in range(NKT):
                lhs = kt[:, j * P:(j + 1) * P]
                c0 = j * P
                pj = p_pool.tile([P, S], fp16, tag="p")
                for c, e in chunks(j):
                    stt = st_psum.tile([P, CHUNK], f32, tag="st")
                    for lo in range(0, e - c, 512):
                        wl = min(512, e - c - lo)
                        nc.tensor.matmul(stt[:, lo:lo + wl], lhsT=lhs,
                                         rhs=qt[:, c + lo:c + lo + wl],
                                         start=True, stop=True)
                    nc.scalar.activation(pj[:, c - c0:e - c0], stt[:, :e - c],
                                         Exp, scale=0.125)
                nc.vector.tensor_mul(pj[:, :S - c0], pj[:, :S - c0],
                                     mask_sb[:, j, c0:])
                if j <= 7:
                    pvA.append((h, outA, vp, j, pj))
                pvB.append((h, outB, vp, j, pj))

                while len(pvA) > PVA_LAG:
                    emit_pvA(*pvA.pop(0))
                if len(pvB) > PVB_HIGH:
                    for _ in range(PVB_BURST):
                        emit_pvB(*pvB.pop(0))
                if MASK_PACE and h == 0 and 5 <= j < 13:
                    load_mask(j + 3)
                if j == 12 and h + 1 < HPC:
                    head_tiles[h + 1] = load_head(h + 1)

        while pvA:
            emit_pvA(*pvA.pop(0))
        while pvB:
            emit_pvB(*pvB.pop(0))
        flush_norms(force=True)

    nc.compile()
    return nc


def prep_inputs(query, key, value, attn_mask):
    """Host-side layout prep (transposes/retiling/casts only) -> 8 in_maps."""
    query = np.asarray(query, dtype=np.float32)
    key = np.asarray(key, dtype=np.float32)
    value = np.asarray(value, dtype=np.float32)
    attn_mask = np.asarray(attn_mask).astype(bool)

    qT = np.ascontiguousarray(query.transpose(0, 1, 3, 2)).astype(np.float16)
    kT = np.ascontiguousarray(key.transpose(0, 1, 3, 2)).astype(np.float16)

    # ones column FIRST: on-device out row 0 = denominator (PSUM partition
    # 0); V lands at partitions 64:128 via the offset DMA in load_head
    vp = np.concatenate(
        [np.ones((B, H, S, 1), np.float32), value], axis=3).astype(np.float16)
    # [B, H, S, 65] -> [B, H, 128, NKT, 65] (partition-contiguous tiles)
    vp = np.ascontiguousarray(
        vp.reshape(B, H, NKT, P, D + 1).transpose(0, 1, 3, 2, 4))

    tril = np.tril(np.ones((S, S), dtype=bool))
    in_maps = []
    for b in range(B):
        m = (attn_mask[b] & tril)          # [q, k]
        mT = m.T.astype(np.float16)        # [k, q]
        maskt = np.ascontiguousarray(
            mT.reshape(NKT, P, S).transpose(1, 0, 2))  # [128, NKT, S]
        for cl in range(NCORES // B):
            h0 = cl * HPC
            in_maps.append({
                "qt": np.ascontiguousarray(qT[b, h0:h0 + HPC]),
                "kt": np.ascontiguousarray(kT[b, h0:h0 + HPC]),
                "vp": np.ascontiguousarray(vp[b, h0:h0 + HPC]),
                "maskt": maskt,
            })
    return in_maps


def run(query, key, value, attn_mask, trace=False, trace_cores=None):
    from concourse import bass_utils

    if "nc" not in _cache:
        _cache["nc"] = build_nc()
    nc = _cache["nc"]

    in_maps = prep_inputs(query, key, value, attn_mask)
    res = bass_utils.run_bass_kernel_spmd(
        nc, in_maps, core_ids=list(range(NCORES)),
        trace=trace, trace_cores=trace_cores)

    out = np.empty((B, H, S, D), np.float32)
    for c in range(NCORES):
        b = c // (NCORES // B)
        h0 = (c % (NCORES // B)) * HPC
        outt = res.results[c]["outt"]          # [HPC, 64, S] fp16
        out[b, h0:h0 + HPC] = outt.astype(np.float32).transpose(0, 2, 1)
    return out, res


def kernel(query, key, value, attn_mask):
    out, _ = run(query, key, value, attn_mask)
    return out


# revision 21
# speedup vs baseline: 1.1373x; 1.1373x over previous
# Trainium2 Bass kernel for masked causal attention
#   B=2, H=16, S=2048, D=64, bool attn_mask [B, S, S] + causal, softmax, @V.
#
# Sharding: 8 cores x 4 heads (cores 0-3 -> batch 0, cores 4-7 -> batch 1).
# Each core computes its 4 heads fully on-device; the per-batch mask is
# resident in SBUF and shared by the core's 4 heads.
#
# Per (head, k-tile j of 128 keys):
#   S^T[k, q] = sum_d K[k,d] Q[q,d]     (PE: lhsT=K^T tile, rhs=Q^T, fp16)
#   p[k, q]   = exp(S^T/8) * mask^T     (ACT exp from PSUM -> fp16 SBUF; DVE mult)
#   outT[m,q] += sum_k vp[k,m] p[k,q]   (PE: lhsT=[ones | V] -> row 0 = denom)
# then outT[1:65]/denom via DVE reciprocal (straight from PSUM row 0) +
# DRAM-roundtrip DMA partition-broadcast + DVE mult, fp16 out DMA.
#
# Structure notes (performance):
# - causal trim everywhere: k-tile j only computes q >= 128j, and the mask
#   DMA only loads those columns (DMA saturation degrades the PE clock).
# - outp is split into two [65, 1024] halves (2 PSUM banks each, pool
#   bufs=2): the A half (cols < 1024) takes PV only from j <= 7, finishes
#   mid-head, and frees its banks early so the next head's PV never stalls
#   on a full-head drain.
# - PV work is queued and drained in dense bursts (the PE p-state decays
#   to 1.2 GHz unless it gets long gapless runs).
# - NOTE: ROW_TILE (paired row-group QK on disjoint PE row halves) hangs
#   the device (NRT_EXEC_UNIT_UNRECOVERABLE) despite passing CoreSim.

import os
import numpy as np

B, H, S, D = 2, 16, 2048, 64
NCORES = 8
HPC = 4          # heads per core
P = 128
NKT = S // P     # 16 k-tiles
CHUNK = 1024     # S^T psum tile width (2 PSUM banks; bufs=2 double-buffers)
ST_BUFS = 2
HALF = 1024      # outp half width
VOFF = 64        # V block offset in vp: [ones | pad(63) | V] -> numerator
                 # starts at PSUM partition 64 (a 64-partition DVE access
                 # must begin at partition 0 or 64)
VPW = VOFF + D   # vp width (96)
PVA_LAG = 2      # prompt queue: PV-A trails QK by this many k-tiles
MASK_PACE = False
DMA_BC = os.environ.get("ATTN_DMA_BC", "0") == "1"
WARM_N = 16      # warm-up matmuls (clock ramp cover for the input DMA window)
PVB_HIGH = 8     # lazy queue: drain in bursts of PVB_BURST above this
PVB_BURST = 4
NORM_DEFER = 2   # norm mult flushes this many PV emissions after its chain

_cache = {}


def build_nc():
    import concourse.bacc as bacc
    import concourse.mybir as mybir
    import concourse.tile as tile
    from concourse import library_config
    from contextlib import ExitStack

    fp16 = mybir.dt.float16
    f32 = mybir.dt.float32
    Exp = mybir.ActivationFunctionType.Exp

    nc = bacc.Bacc("TRN2", target_bir_lowering=False, debug=False,
                   num_devices=NCORES)

    qt_d = nc.dram_tensor("qt", [HPC, 64, S], fp16, kind="ExternalInput")
    kt_d = nc.dram_tensor("kt", [HPC, 64, S], fp16, kind="ExternalInput")
    vp_d = nc.dram_tensor("vp", [HPC, P, NKT, D + 1], fp16, kind="ExternalInput")
    mk_d = nc.dram_tensor("maskt", [P, NKT, S], fp16, kind="ExternalInput")
    out_d = nc.dram_tensor("outt", [HPC, D, S], fp16, kind="ExternalOutput")

    with tile.TileContext(nc) as tc, ExitStack() as ctx:
        mask_pool = ctx.enter_context(tc.tile_pool(name="mask", bufs=1))
        qk_pool = ctx.enter_context(tc.tile_pool(name="qk", bufs=2))
        vp_pool = ctx.enter_context(tc.tile_pool(name="vpool", bufs=2))
        p_pool = ctx.enter_context(tc.tile_pool(name="p", bufs=max(12, PVB_HIGH + 4)))
        o_pool = ctx.enter_context(tc.tile_pool(name="osb", bufs=3))
        r_pool = ctx.enter_context(tc.tile_pool(name="recip", bufs=4))
        warm_pool = ctx.enter_context(tc.tile_pool(name="warm", bufs=1))
        rb_pool = ctx.enter_context(tc.tile_pool(name="rb", bufs=4))
        st_psum = ctx.enter_context(tc.tile_pool(name="st", bufs=ST_BUFS, space="PSUM"))
        o_psum = ctx.enter_context(tc.tile_pool(name="outp", bufs=2, space="PSUM"))
        dram_pool = ctx.enter_context(tc.tile_pool(name="dram", bufs=4, space="DRAM"))

        if not DMA_BC:
            nc.gpsimd.load_library(library_config.attn)
        # PE warm-up: dense back-to-back matmuls on zeros right at kernel
        # start so the clock-gate opens to 2.4 GHz before the real QK stream.
        wsb = warm_pool.tile([P, 512], fp16, tag="warm")
        if os.environ.get("ATTN_WARM_MEMSET", "1") == "1":
            nc.vector.memset(wsb[:], 0.0)
        wps = st_psum.tile([P, CHUNK], f32, tag="st")
        for i in range(WARM_N):
            lo = 512 * (i % 2)
            nc.tensor.matmul(wps[:, lo:lo + 512], lhsT=wsb[:, 0:128],
                             rhs=wsb[:], start=True, stop=True)

        def load_head(h):
            qt = qk_pool.tile([64, S], fp16, tag="qt")
            nc.sync.dma_start(qt[:], qt_d[h])
            kt = qk_pool.tile([64, S], fp16, tag="kt")
            nc.sync.dma_start(kt[:], kt_d[h])
            vp = vp_pool.tile([P, NKT, VPW], fp16, tag="vp")
            # ones col -> 0, V -> 64:128; pad cols 1:64 stay garbage (they
            # only feed PSUM partitions 1:63, which are never read)
            nc.sync.dma_start(vp[:, :, 0:1], vp_d[h, :, :, 0:1])
            nc.sync.dma_start(vp[:, :, VOFF:VPW], vp_d[h, :, :, 1:D + 1])
            return qt, kt, vp

        # Head 0 inputs first (unblocks the first QK ~4us in), then the big
        # per-batch mask^T streams in behind it, causally trimmed.
        head_tiles = {0: load_head(0)}
        mask_sb = mask_pool.tile([P, NKT, S], fp16, tag="mask")

        def load_mask(g):
            nc.sync.dma_start(mask_sb[:, g:g + 1, g * P:],
                              mk_d[:, g:g + 1, g * P:])

        for g in range(8 if MASK_PACE else NKT):
            load_mask(g)

        pvA = []          # (h, outA, vp, j, pj) — prompt, j <= 7 only
        pvB = []          # (h, outB, vp, j, pj) — lazy burst reserve
        norm_pend = []    # (due_emit_count, h, outX, s0, rbc)
        n_emitted = [0]

        def flush_norms(force=False):
            while norm_pend and (force or norm_pend[0][0] <= n_emitted[0]):
                _, h, outX, s0, rbc = norm_pend.pop(0)
                lo = s0 % HALF
                osb = o_pool.tile([D, 512], fp16, tag="osb")
                nc.vector.tensor_mul(osb[:], outX[VOFF:VOFF + D, lo:lo + 512], rbc[:])
                nc.sync.dma_start(out_d[h, :, s0:s0 + 512], osb[:])

        def emit_norm(h, outX, s0):
            # denominator is row 0 of outX (ones-first vp), read from PSUM
            # directly (base partition 0, so reciprocal_approx_fast is safe).
            lo = s0 % HALF
            recip = r_pool.tile([1, 512], f32, tag="recip")
            nc.vector.reciprocal_approx_fast(out=recip[0:1, :],
                                             in_=outX[0:1, lo:lo + 512])
            # partition-broadcast via a DRAM round trip (2 small DMAs):
            # avoids the gpsimd library entirely (its TENSOR_LOAD serializes
            # ~2us of kernel startup) and keeps the Pool engine idle.
            rbc = rb_pool.tile([D, 512], f32, tag="rbc")
            if DMA_BC:
                rscr = dram_pool.tile([1, 512], f32, tag="rscr")
                nc.sync.dma_start(rscr[0:1, :], recip[0:1, :])
                nc.sync.dma_start(rbc[:], rscr[0:1, :].to_broadcast((D, 512)))
            else:
                nc.gpsimd.partition_broadcast(rbc[:], recip[0:1, :])
            norm_pend.append((n_emitted[0] + NORM_DEFER, h, outX, s0, rbc))

        def emit_pvA(h, outA, vp, j, pj):
            flush_norms()
            c = j * P
            for b in range(c // 512, 2):
                g0, g1 = max(c, 512 * b), 512 * (b + 1)
                nc.tensor.matmul(outA[:, g0:g1], lhsT=vp[:, j, :],
                                 rhs=pj[:, g0 - c:g1 - c],
                                 start=(j == 0),
                                 stop=(j == min(4 * b + 3, 7)))
            n_emitted[0] += 1
            if j == 3:
                emit_norm(h, outA, 0)
            elif j == 7:
                emit_norm(h, outA, 512)

        def emit_pvB(h, outB, vp, j, pj):
            if j == 0:
                # outB reuses the previous head's B banks; its norm mults
                # must be on the DVE queue before the PE waits on the slot.
                flush_norms(force=True)
            else:
                flush_norms()
            c = j * P
            for b in range(max(c - HALF, 0) // 512, 2):
                g0 = max(c, HALF + 512 * b)
                g1 = HALF + 512 * (b + 1)
                nc.tensor.matmul(outB[:, g0 - HALF:g1 - HALF],
                                 lhsT=vp[:, j, :], rhs=pj[:, g0 - c:g1 - c],
                                 start=(j == 0),
                                 stop=(j == min(4 * b + 11, NKT - 1)))
            n_emitted[0] += 1
            if j == 11:
                emit_norm(h, outB, HALF)
            elif j == 15:
                emit_norm(h, outB, HALF + 512)

        def chunks(j):
            out, c = [], j * P
            while c < S:
                e = min(S, (c // CHUNK + 1) * CHUNK)
                out.append((c, e))
                c = e
            return out

        for h in range(HPC):
            qt, kt, vp = head_tiles.pop(h, None) or load_head(h)
            outA = o_psum.tile([VPW, HALF], f32, tag="outp")
            outB = o_psum.tile([VPW, HALF], f32, tag="outp")

            for j in range(NKT):
                lhs = kt[:, j * P:(j + 1) * P]
                c0 = j * P
                pj = p_pool.tile([P, S], fp16, tag="p")
                for c, e in chunks(j):
                    stt = st_psum.tile([P, CHUNK], f32, tag="st")
                    for lo in range(0, e - c, 512):
                        wl = min(512, e - c - lo)
                        nc.tensor.matmul(stt[:, lo:lo + wl], lhsT=lhs,
                                         rhs=qt[:, c + lo:c + lo + wl],
                                         start=True, stop=True)
                    nc.scalar.activation(pj[:, c - c0:e - c0], stt[:, :e - c],
                                         Exp, scale=0.125)
                nc.vector.tensor_mul(pj[:, :S - c0], pj[:, :S - c0],
                                     mask_sb[:, j, c0:])
                if j <= 7:
                    pvA.append((h, outA, vp, j, pj))
                pvB.append((h, outB, vp, j, pj))

                while len(pvA) > PVA_LAG:
                    emit_pvA(*pvA.pop(0))
                if len(pvB) > PVB_HIGH:
                    for _ in range(PVB_BURST):
                        emit_pvB(*pvB.pop(0))
                if MASK_PACE and h == 0 and 5 <= j < 13:
                    load_mask(j + 3)
                if j == 12 and h + 1 < HPC:
                    head_tiles[h + 1] = load_head(h + 1)

        while pvA:
            emit_pvA(*pvA.pop(0))
        while pvB:
            emit_pvB(*pvB.pop(0))
        flush_norms(force=True)

    nc.compile()
    return nc


def prep_inputs(query, key, value, attn_mask):
    """Host-side layout prep (transposes/retiling/casts only) -> 8 in_maps."""
    query = np.asarray(query, dtype=np.float32)
    key = np.asarray(key, dtype=np.float32)
    value = np.asarray(value, dtype=np.float32)
    attn_mask = np.asarray(attn_mask).astype(bool)

    qT = np.ascontiguousarray(query.transpose(0, 1, 3, 2)).astype(np.float16)
    kT = np.ascontiguousarray(key.transpose(0, 1, 3, 2)).astype(np.float16)

    # ones column FIRST: on-device out row 0 = denominator (PSUM partition
    # 0); V lands at partitions 64:128 via the offset DMA in load_head
    vp = np.concatenate(
        [np.ones((B, H, S, 1), np.float32), value], axis=3).astype(np.float16)
    # [B, H, S, 65] -> [B, H, 128, NKT, 65] (partition-contiguous tiles)
    vp = np.ascontiguousarray(
        vp.reshape(B, H, NKT, P, D + 1).transpose(0, 1, 3, 2, 4))

    tril = np.tril(np.ones((S, S), dtype=bool))
    in_maps = []
    for b in range(B):
        m = (attn_mask[b] & tril)          # [q, k]
        mT = m.T.astype(np.float16)        # [k, q]
        maskt = np.ascontiguousarray(
            mT.reshape(NKT, P, S).transpose(1, 0, 2))  # [128, NKT, S]
        for cl in range(NCORES // B):
            h0 = cl * HPC
            in_maps.append({
                "qt": np.ascontiguousarray(qT[b, h0:h0 + HPC]),
                "kt": np.ascontiguousarray(kT[b, h0:h0 + HPC]),
                "vp": np.ascontiguousarray(vp[b, h0:h0 + HPC]),
                "maskt": maskt,
            })
    return in_maps


def run(query, key, value, attn_mask, trace=False, trace_cores=None):
    from concourse import bass_utils

    if "nc" not in _cache:
        _cache["nc"] = build_nc()
    nc = _cache["nc"]

    in_maps = prep_inputs(query, key, value, attn_mask)
    res = bass_utils.run_bass_kernel_spmd(
        nc, in_maps, core_ids=list(range(NCORES)),
        trace=trace, trace_cores=trace_cores)

    out = np.empty((B, H, S, D), np.float32)
    for c in range(NCORES):
        b = c // (NCORES // B)
        h0 = (c % (NCORES // B)) * HPC
        outt = res.results[c]["outt"]          # [HPC, 64, S] fp16
        out[b, h0:h0 + HPC] = outt.astype(np.float32).transpose(0, 2, 1)
    return out, res


def kernel(query, key, value, attn_mask):
    out, _ = run(query, key, value, attn_mask)
    return out
